# revision 1
# baseline (speedup 1.0000x reference)
"""Trainium2 Bass kernel for nn_DiffusionLM (dense_mlp).

Strategy (8 NeuronCores, data-parallel over tokens):
  - 4096 tokens total -> 512 tokens per core; params replicated.
  - Host prep: embedding gather h0 = embed[x], per-step scalar constants,
    step-bias table r1[t] = t_norm[t]*W1[512,:] + b1, and the vocab head
    pre-transposed/gain-folded and cast to bf16: embt = bf16((embed*gn).T).
  - Device per core: 20 reverse-diffusion steps of a 3-layer MLP
    (matmul + LayerNorm + exact gelu) with all activations SBUF-resident,
    token-major layout ([128 tokens, features]); PE transposes feed each
    matmul's stationary operand.  Matmuls run as float32r (full PE rate).
    LayerNorm rstd uses a DVE-side bit-trick rsqrt (keeps the scalar
    engine's activation table pinned to the gelu set).  PSUM tiles are all
    single-bank with a deep shared rotation so the PE never waits on a
    LayerNorm chain.  Then final LayerNorm and the [512,512] @ [512,32000]
    vocab projection in bf16, streaming embt from HBM.
  - Host: concatenate per-core logits -> [2,2048,32000] fp32.
"""

import numpy as np
import ml_dtypes

import concourse.bass as bass
import concourse.mybir as mybir
import concourse.tile as tile
from concourse import bacc, bass_utils
from concourse.bass import ds, ts
from concourse.masks import make_identity

dt = mybir.dt
F32 = dt.float32
F32R = dt.float32r
BF16 = dt.bfloat16
I32 = dt.int32
AF = mybir.ActivationFunctionType
ALU = mybir.AluOpType

# Problem shapes (hardcoded per contract; kernel.py must be self-contained).
N_CORES = 8
VOCAB = 32000
HID = 512
DH = 2 * HID  # 1024
N_STEPS = 20
EPS = 1e-5
B, S = 2, 2048
T_TOTAL = B * S              # 4096
T_CORE = T_TOTAL // N_CORES  # 512
P = 128                      # partitions
RSQRT_MAGIC = 0x5F3759DF


def _step_consts(n_steps):
    """Per-step scalars, ordered t = n_steps-1 .. 0, matching reference."""
    betas = np.linspace(0.0001, 0.02, n_steps, dtype=np.float32)
    alphas = (1.0 - betas).astype(np.float32)
    acp = np.cumprod(alphas, dtype=np.float32)
    tsx = np.arange(n_steps - 1, -1, -1)
    t_norm = (tsx.astype(np.float32) / np.float32(n_steps)).astype(np.float32)
    coef = (betas[tsx] / np.sqrt((np.float32(1.0) - acp[tsx]))).astype(np.float32)
    isa = (np.float32(1.0) / np.sqrt(alphas[tsx])).astype(np.float32)
    return t_norm, coef, isa


def build_program(t_core=T_CORE, n_steps=N_STEPS, vocab=VOCAB,
                  apply_gb1=False, apply_gb2=False,
                  use_b2=False, use_b3=False, use_voff=False):
    """Trace + compile the Bass/Tile program. Returns nc."""
    tp_n = t_core // P  # token tiles per core

    nc = bacc.Bacc("TRN2", target_bir_lowering=False, debug=False,
                   num_devices=N_CORES)

    h0_d = nc.dram_tensor("h0", [t_core, HID], F32, kind="ExternalInput").ap()
    w1_d = nc.dram_tensor("w1", [HID, DH], F32, kind="ExternalInput").ap()
    r1_d = nc.dram_tensor("r1", [1, n_steps, DH], F32,
                          kind="ExternalInput").ap()
    w2_d = nc.dram_tensor("w2", [DH, DH], F32, kind="ExternalInput").ap()
    w3_d = nc.dram_tensor("w3", [DH, HID], F32, kind="ExternalInput").ap()
    emb_d = nc.dram_tensor("embt", [HID, vocab], BF16,
                           kind="ExternalInput").ap()
    out_d = nc.dram_tensor("logits", [t_core, vocab], F32,
                           kind="ExternalOutput").ap()
    b2_d = b3_d = voff_d = None
    if use_b2:
        b2_d = nc.dram_tensor("b2", [1, DH], F32, kind="ExternalInput").ap()
    if use_b3:
        b3_d = nc.dram_tensor("b3", [1, HID], F32, kind="ExternalInput").ap()
    if use_voff:
        voff_d = nc.dram_tensor("voff", [1, vocab], F32,
                                kind="ExternalInput").ap()
    gb_d = None
    if apply_gb1 or apply_gb2:
        gb_d = nc.dram_tensor("gb", [4, DH], F32, kind="ExternalInput").ap()

    t_norm, coef, isa = _step_consts(n_steps)

    def r_(ap):  # f32r view for diffusion matmul operands
        return ap.bitcast(F32R)

    with tile.TileContext(nc) as tc:
      with (
          tc.tile_pool(name="wpool", bufs=1) as wpool,
          tc.tile_pool(name="work", bufs=3) as work,
          tc.tile_pool(name="emb", bufs=4) as embp,
          tc.tile_pool(name="lout", bufs=4) as loutp,
          tc.tile_pool(name="ps", bufs=6, space="PSUM") as psp,
      ):
            # ---- resident constants/weights ----
            ident = wpool.tile([P, P], F32)
            make_identity(nc, ident)
            ones1 = wpool.tile([1, P], F32)
            ones1_f = wpool.tile([1, P], F32, name="ones1_f")
            nc.vector.memset(ones1_f, 1.0)
            nc.vector.tensor_copy(out=r_(ones1), in_=ones1_f)
            magict = wpool.tile([P, 1], I32)
            nc.vector.memset(magict, RSQRT_MAGIC)

            w1s = []
            for kc in range(HID // P):
                w = wpool.tile([P, DH], F32, tag=f"w1_{kc}")
                nc.sync.dma_start(out=r_(w),
                                  in_=r_(w1_d[kc * P:(kc + 1) * P, :]))
                w1s.append(w)
            w2s = []
            for kc in range(DH // P):
                w = wpool.tile([P, DH], F32, tag=f"w2_{kc}")
                nc.sync.dma_start(out=r_(w),
                                  in_=r_(w2_d[kc * P:(kc + 1) * P, :]))
                w2s.append(w)
            w3s = []
            for kc in range(DH // P):
                w = wpool.tile([P, HID], F32, tag=f"w3_{kc}")
                nc.sync.dma_start(out=r_(w),
                                  in_=r_(w3_d[kc * P:(kc + 1) * P, :]))
                w3s.append(w)
            b2s = b3s = voff_s = None
            if use_b2:
                b2s = wpool.tile([1, DH], F32)
                nc.sync.dma_start(out=r_(b2s), in_=r_(b2_d))
            if use_b3:
                b3s = wpool.tile([1, HID], F32)
                nc.sync.dma_start(out=r_(b3s), in_=r_(b3_d))
            if use_voff:
                voff_s = wpool.tile([1, vocab], F32)
                nc.sync.dma_start(out=voff_s, in_=voff_d)
            gbs = None
            if gb_d is not None:
                gbs = wpool.tile([P, 4, DH], F32)
                nc.sync.dma_start(out=gbs, in_=gb_d.to_broadcast([P, 4, DH]))

            hs = []
            for tp in range(tp_n):
                h = wpool.tile([P, HID], F32, tag=f"h_{tp}")
                nc.sync.dma_start(out=h, in_=h0_d[tp * P:(tp + 1) * P, :])
                hs.append(h)
            hcTs = [wpool.tile([P, HID], BF16, tag=f"hcT_{tp}",
                               name=f"hcT_{tp}") for tp in range(tp_n)]

            n_evac = [0]

            def evac(dst, src):
                """PSUM->SBUF copy, rotating between DVE and ACT (2:1)."""
                if n_evac[0] % 3 == 2:
                    nc.scalar.copy(out=dst, in_=src)
                else:
                    nc.vector.tensor_copy(out=dst, in_=src)
                n_evac[0] += 1

            def rsqrt_dve(y, u, tmp, n_iter=2):
                """y = 1/sqrt(u) via bit-trick + Newton, all on DVE.

                y/u/tmp are [P,1] f32 APs."""
                nc.vector.tensor_scalar(out=y.bitcast(I32),
                                        in0=u.bitcast(I32), scalar1=1,
                                        scalar2=None,
                                        op0=ALU.logical_shift_right)
                nc.vector.tensor_tensor(out=y.bitcast(I32), in0=magict,
                                        in1=y.bitcast(I32), op=ALU.subtract)
                for _ in range(n_iter):
                    nc.vector.tensor_tensor(out=tmp, in0=y, in1=y,
                                            op=ALU.mult)
                    nc.vector.tensor_tensor(out=tmp, in0=tmp, in1=u,
                                            op=ALU.mult)
                    nc.vector.tensor_scalar(out=tmp, in0=tmp, scalar1=-0.5,
                                            scalar2=1.5, op0=ALU.mult,
                                            op1=ALU.add)
                    nc.vector.tensor_tensor(out=y, in0=y, in1=tmp,
                                            op=ALU.mult)

            def layernorm_gelu(y, width, gb_idx):
                """In-place y[SBUF f32] <- gelu(LN(y) (*g+be))."""
                nparts = width // 512
                st = work.tile([P, nparts, 6], F32, tag="st")
                for i in range(nparts):
                    nc.vector.bn_stats(out=st[:, i, :],
                                       in_=y[:, ds(i * 512, 512)])
                mv = work.tile([P, 2], F32, tag="mv")
                nc.vector.bn_aggr(out=mv, in_=st)
                # rstd = rsqrt(var + eps) on DVE (no ACT table swap)
                sc2 = work.tile([P, 3], F32, tag="sc2")
                u, rstd, tmp = sc2[:, 0:1], sc2[:, 1:2], sc2[:, 2:3]
                nc.vector.tensor_scalar(out=u, in0=mv[:, 1:2], scalar1=EPS,
                                        scalar2=None, op0=ALU.add)
                rsqrt_dve(rstd, u, tmp)
                for i in range(nparts):
                    nc.vector.tensor_scalar(
                        out=y[:, ds(i * 512, 512)], in0=y[:, ds(i * 512, 512)],
                        scalar1=mv[:, 0:1], scalar2=rstd,
                        op0=ALU.subtract, op1=ALU.mult)
                if gb_idx is not None and gbs is not None:
                    g_t, be_t = gbs[:, gb_idx, :], gbs[:, gb_idx + 1, :]
                    nc.vector.tensor_mul(out=y, in0=y, in1=g_t[:, :width])
                    nc.vector.tensor_add(out=y, in0=y, in1=be_t[:, :width])
                for i in range(nparts):
                    nc.scalar.activation(out=y[:, ds(i * 512, 512)],
                                         in_=y[:, ds(i * 512, 512)],
                                         func=AF.Gelu)

            def transpose_to(src, width, out_dt=F32):
                """SBUF tile [P,width](out_dt) = 128-block transposes of src.

                Uses [P,512] single-bank PSUM tiles."""
                out = work.tile([P, width], out_dt,
                                tag="hT" if width <= 512 else "zT")
                for g in range(width // 512):
                    pt = psp.tile([P, 512], F32, tag="ps")
                    for kc in range(4):
                        c = g * 4 + kc
                        nc.tensor.transpose(pt[:, ts(kc, P)],
                                            src[:, ts(c, P)], ident)
                    dst = out[:, ds(g * 512, 512)]
                    evac(dst if out_dt == BF16 else r_(dst), pt)
                return out

            # ================= diffusion =================
            # Stage-interleaved emission: each engine's program order
            # alternates between the 4 token tiles, so the in-order PE
            # queue always has another tile's matmuls while a LayerNorm
            # chain completes on DVE/ACT.
            def evac_y(y, pps):
                """Stage PSUM halves into SBUF tile y, freeing PSUM early."""
                for i, pp in enumerate(pps):
                    evac(y[:, ds(i * 512, 512)], pp)

            def mlp_layer(xT, ws, n_k, extra):
                """PSUM halves of xT.T @ W (+ optional K=1 extra rows)."""
                pps = []
                for i in range(2):
                    sl = ds(i * 512, 512)
                    pp = psp.tile([P, 512], F32, tag="ps")
                    for kc in range(n_k):
                        nc.tensor.matmul(pp, r_(xT[:, ts(kc, P)]),
                                         r_(ws[kc][:, sl]),
                                         start=(kc == 0),
                                         stop=(kc == n_k - 1 and not extra))
                    for row, tab in (extra or []):
                        nc.tensor.matmul(pp, r_(row), r_(tab[:, sl]),
                                         start=False, stop=True)
                    pps.append(pp)
                return pps

            for step in range(n_steps):
                c_isa = float(coef[step] * isa[step])
                isa_f = float(isa[step])
                r1row = work.tile([1, DH], F32, tag="r1row", bufs=2,
                                  name=f"r1row_{step}")
                nc.sync.dma_start(out=r_(r1row), in_=r_(r1_d[:, step, :]))
                z1s, z2s = {}, {}
                for tp in range(tp_n):
                    hT = transpose_to(hs[tp], HID)
                    pps = mlp_layer(hT, w1s, 4, [(ones1, r1row)])
                    z1 = work.tile([P, DH], F32, tag="zn", name=f"z1_{tp}")
                    evac_y(z1, pps)
                    z1s[tp] = z1
                for tp in range(tp_n):
                    layernorm_gelu(z1s[tp], DH, 0 if apply_gb1 else None)
                    z1T = transpose_to(z1s[tp], DH)
                    extra2 = [(ones1, b2s)] if use_b2 else None
                    pps = mlp_layer(z1T, w2s, 8, extra2)
                    z2 = work.tile([P, DH], F32, tag="zn", name=f"z2_{tp}")
                    evac_y(z2, pps)
                    z2s[tp] = z2
                for tp in range(tp_n):
                    layernorm_gelu(z2s[tp], DH, 2 if apply_gb2 else None)
                    z2T = transpose_to(z2s[tp], DH)
                    ps3 = psp.tile([P, 512], F32, tag="ps")
                    for kc in range(8):
                        last = (kc == 7) and not use_b3
                        nc.tensor.matmul(ps3, r_(z2T[:, ts(kc, P)]),
                                         r_(w3s[kc]),
                                         start=(kc == 0), stop=last)
                    if use_b3:
                        nc.tensor.matmul(ps3, r_(ones1), r_(b3s),
                                         start=False, stop=True)
                    # h = isa*h - (coef*isa)*score
                    h = hs[tp]
                    sc = work.tile([P, HID], F32, tag="sc")
                    nc.vector.tensor_scalar(out=sc, in0=ps3, scalar1=c_isa,
                                            scalar2=None, op0=ALU.mult)
                    hm = work.tile([P, HID], F32, tag="hm")
                    nc.scalar.mul(hm, h, isa_f)
                    nc.gpsimd.tensor_tensor(out=h, in0=hm, in1=sc,
                                            op=ALU.subtract)

            # ============ final LN + hcT (bf16) ============
            for tp in range(tp_n):
                h = hs[tp]
                st = work.tile([P, 6], F32, tag="stf")
                nc.vector.bn_stats(out=st, in_=h)
                mv = work.tile([P, 2], F32, tag="mv")
                nc.vector.bn_aggr(out=mv, in_=st)
                sc2 = work.tile([P, 3], F32, tag="sc2")
                u, rstd, tmp = sc2[:, 0:1], sc2[:, 1:2], sc2[:, 2:3]
                nc.vector.tensor_scalar(out=u, in0=mv[:, 1:2], scalar1=EPS,
                                        scalar2=None, op0=ALU.add)
                rsqrt_dve(rstd, u, tmp)
                hc = work.tile([P, HID], F32, tag="hm")
                nc.vector.tensor_scalar(out=hc, in0=h, scalar1=mv[:, 0:1],
                                        scalar2=rstd, op0=ALU.subtract,
                                        op1=ALU.mult)
                pt = psp.tile([P, 512], F32, tag="ps")
                for kc in range(4):
                    nc.tensor.transpose(pt[:, ts(kc, P)], hc[:, ts(kc, P)],
                                        ident)
                nc.vector.tensor_copy(out=hcTs[tp], in_=pt)

            # ================= logits (bf16) =================
            VC = 2048  # vocab stream chunk (bf16 -> 4KB/partition rows)
            n_vc = (vocab + VC - 1) // VC
            n_out = 0
            for vc in range(n_vc):
                v0 = vc * VC
                vn = min(VC, vocab - v0)
                et = embp.tile([P, 4, vn], BF16, tag="et")
                for kc in range(4):
                    nc.sync.dma_start(
                        out=et[:, kc, :],
                        in_=emb_d[kc * P:(kc + 1) * P, v0:v0 + vn])
                for tp in range(tp_n):
                    for i in range((vn + 511) // 512):
                        w = min(512, vn - i * 512)
                        pl = psp.tile([P, 512], F32, tag="ps")
                        for kc in range(4):
                            last = (kc == 3) and not use_voff
                            nc.tensor.matmul(
                                pl[:, :w], hcTs[tp][:, ts(kc, P)],
                                et[:, kc, ds(i * 512, w)],
                                start=(kc == 0), stop=last)
                        if use_voff:
                            nc.tensor.matmul(
                                pl[:, :w], r_(ones1),
                                r_(voff_s[:, ds(v0 + i * 512, w)]),
                                start=False, stop=True)
                        lo = loutp.tile([P, 512], F32, tag="lo")
                        if n_out % 3 == 0:
                            nc.vector.tensor_copy(out=lo[:, :w], in_=pl[:, :w])
                        else:
                            nc.scalar.copy(out=lo[:, :w], in_=pl[:, :w])
                        n_out += 1
                        nc.sync.dma_start(
                            out=out_d[tp * P:(tp + 1) * P,
                                      v0 + i * 512:v0 + i * 512 + w],
                            in_=lo[:, :w])
    nc.compile()
    return nc


def host_prep(x, embed, W1, b1, g1, be1, W2, b2, g2, be2, W3, b3, gn, bn,
              n_steps=N_STEPS):
    """Pure-numpy input prep shared by all cores."""
    x = np.asarray(x).reshape(-1)
    embed = np.asarray(embed, dtype=np.float32)
    W1 = np.asarray(W1, dtype=np.float32)
    b1 = np.asarray(b1, dtype=np.float32)
    t_norm, _, _ = _step_consts(n_steps)
    h0 = embed[x]                                     # [T_total, HID]
    r1 = (t_norm[:, None] * W1[HID][None, :]
          + b1[None, :]).astype(np.float32)[None]
    gnf = np.asarray(gn, dtype=np.float32)
    embt = np.ascontiguousarray(
        (embed * gnf[None, :]).T.astype(ml_dtypes.bfloat16))  # [HID, VOCAB]
    voff = (np.asarray(bn, dtype=np.float32) @ embed.T).astype(np.float32)
    return dict(
        h0=h0,
        w1=np.ascontiguousarray(W1[:HID]),
        r1=r1,
        w2=np.asarray(W2, dtype=np.float32),
        w3=np.asarray(W3, dtype=np.float32),
        embt=embt,
        b2=np.asarray(b2, dtype=np.float32).reshape(1, -1),
        b3=np.asarray(b3, dtype=np.float32).reshape(1, -1),
        voff=voff.reshape(1, -1),
        g1=np.asarray(g1, dtype=np.float32),
        be1=np.asarray(be1, dtype=np.float32),
        g2=np.asarray(g2, dtype=np.float32),
        be2=np.asarray(be2, dtype=np.float32),
    )


_CACHE = {}


def _get_program(key, **kw):
    if key not in _CACHE:
        _CACHE[key] = build_program(**kw)
    return _CACHE[key]


def kernel(x, embed, W1, b1, g1, be1, W2, b2, g2, be2, W3, b3, gn, bn,
           run_kwargs=None):
    pre = host_prep(x, embed, W1, b1, g1, be1, W2, b2, g2, be2, W3, b3,
                    gn, bn)

    apply_gb1 = bool(np.any(pre["g1"] != 1.0) or np.any(pre["be1"] != 0.0))
    apply_gb2 = bool(np.any(pre["g2"] != 1.0) or np.any(pre["be2"] != 0.0))
    use_b2 = bool(np.any(pre["b2"]))
    use_b3 = bool(np.any(pre["b3"]))
    use_voff = bool(np.any(pre["voff"]))

    key = (apply_gb1, apply_gb2, use_b2, use_b3, use_voff)
    nc = _get_program(key, apply_gb1=apply_gb1, apply_gb2=apply_gb2,
                      use_b2=use_b2, use_b3=use_b3, use_voff=use_voff)

    common = {"w1": pre["w1"], "r1": pre["r1"], "w2": pre["w2"],
              "w3": pre["w3"], "embt": pre["embt"]}
    if use_b2:
        common["b2"] = pre["b2"]
    if use_b3:
        common["b3"] = pre["b3"]
    if use_voff:
        common["voff"] = pre["voff"]
    if apply_gb1 or apply_gb2:
        common["gb"] = np.stack([pre["g1"], pre["be1"], pre["g2"],
                                 pre["be2"]])

    in_maps = []
    for c in range(N_CORES):
        m = dict(common)
        m["h0"] = np.ascontiguousarray(pre["h0"][c * T_CORE:(c + 1) * T_CORE])
        in_maps.append(m)

    res = bass_utils.run_bass_kernel_spmd(
        nc, in_maps, core_ids=list(range(N_CORES)), **(run_kwargs or {}))
    out = np.concatenate([res.results[c]["logits"] for c in range(N_CORES)],
                         axis=0)
    kernel.last_results = res
    return out.reshape(B, S, VOCAB)



# revision 8
# speedup vs baseline: 1.0933x; 1.0933x over previous
"""Trainium2 Bass kernel for nn_DiffusionLM (dense_mlp).

Strategy (8 NeuronCores, data-parallel over tokens):
  - 4096 tokens total -> 512 tokens per core; params replicated (bf16).
  - Host prep: embedding gather h0 = embed[x] (f32), step-bias table
    r1[t] = t_norm[t]*W1[512,:] + b1 (bf16), weights bf16, vocab head
    pre-transposed/gain-folded bf16: embt = bf16((embed*gn).T).
  - Device per core: 20 reverse-diffusion steps of a 3-layer MLP in bf16
    (matmul + LayerNorm + exact gelu), token-major activations
    [128 tokens, features]; PE transposes (bf16 identity, 1 cyc/row) feed
    each matmul's stationary operand.  LayerNorm normalize+gelu+PSUM-evac
    are fused into a single ACT pass per 512 cols using per-partition
    scale=rstd / bias=-mu*rstd operands; rstd comes from a DVE-side
    bit-trick rsqrt (no ACT table swap).  The h update
    h' = isa*h - c*isa*score runs split across DVE/ACT/GPSIMD.
  - Final LN + [512,512] @ [512,32000] vocab projection in bf16, streaming
    embt from HBM (prefetched during diffusion); logits stored bf16.
  - Host: concatenate per-core logits, cast f32 -> [2,2048,32000].
"""

import numpy as np
import ml_dtypes

import concourse.bass as bass
import concourse.mybir as mybir
import concourse.tile as tile
from concourse import bacc, bass_utils
from concourse.bass import ds, ts
from concourse.masks import make_identity

dt = mybir.dt
F32 = dt.float32
F32R = dt.float32r
BF16 = dt.bfloat16
I32 = dt.int32
AF = mybir.ActivationFunctionType
ALU = mybir.AluOpType

# Problem shapes (hardcoded per contract; kernel.py must be self-contained).
N_CORES = 8
VOCAB = 32000
HID = 512
DH = 2 * HID  # 1024
N_STEPS = 20
EPS = 1e-5
B, S = 2, 2048
T_TOTAL = B * S              # 4096
T_CORE = T_TOTAL // N_CORES  # 512
P = 128                      # partitions
RSQRT_MAGIC = 0x5F3759DF


def _step_consts(n_steps):
    """Per-step scalars, ordered t = n_steps-1 .. 0, matching reference."""
    betas = np.linspace(0.0001, 0.02, n_steps, dtype=np.float32)
    alphas = (1.0 - betas).astype(np.float32)
    acp = np.cumprod(alphas, dtype=np.float32)
    tsx = np.arange(n_steps - 1, -1, -1)
    t_norm = (tsx.astype(np.float32) / np.float32(n_steps)).astype(np.float32)
    coef = (betas[tsx] / np.sqrt((np.float32(1.0) - acp[tsx]))).astype(np.float32)
    isa = (np.float32(1.0) / np.sqrt(alphas[tsx])).astype(np.float32)
    return t_norm, coef, isa


def build_program(t_core=T_CORE, n_steps=N_STEPS, vocab=VOCAB,
                  apply_gb1=False, apply_gb2=False,
                  use_b2=False, use_b3=False, use_voff=False):
    """Trace + compile the Bass/Tile program. Returns nc."""
    tp_n = t_core // P  # token tiles per core

    nc = bacc.Bacc("TRN2", target_bir_lowering=False, debug=False,
                   num_devices=N_CORES)

    h0_d = nc.dram_tensor("h0", [t_core, HID], F32, kind="ExternalInput").ap()
    w1_d = nc.dram_tensor("w1", [HID, DH], BF16, kind="ExternalInput").ap()
    r1_d = nc.dram_tensor("r1", [1, n_steps, DH], BF16,
                          kind="ExternalInput").ap()
    w2_d = nc.dram_tensor("w2", [DH, DH], BF16, kind="ExternalInput").ap()
    w3_d = nc.dram_tensor("w3", [DH, HID], BF16, kind="ExternalInput").ap()
    emb_d = nc.dram_tensor("embt", [HID, vocab], BF16,
                           kind="ExternalInput").ap()
    out_d = nc.dram_tensor("logits", [t_core, vocab], BF16,
                           kind="ExternalOutput").ap()
    b2_d = b3_d = voff_d = None
    if use_b2:
        b2_d = nc.dram_tensor("b2", [1, DH], BF16, kind="ExternalInput").ap()
    if use_b3:
        b3_d = nc.dram_tensor("b3", [1, HID], BF16, kind="ExternalInput").ap()
    if use_voff:
        voff_d = nc.dram_tensor("voff", [1, vocab], BF16,
                                kind="ExternalInput").ap()
    gb_d = None
    if apply_gb1 or apply_gb2:
        gb_d = nc.dram_tensor("gb", [4, DH], F32, kind="ExternalInput").ap()

    t_norm, coef, isa = _step_consts(n_steps)

    with tile.TileContext(nc) as tc:
      with (
          tc.tile_pool(name="wpool", bufs=1) as wpool,
          tc.tile_pool(name="work", bufs=3) as work,
          tc.tile_pool(name="emb", bufs=3) as embp,
          tc.tile_pool(name="lout", bufs=4) as loutp,
          tc.tile_pool(name="ps", bufs=6, space="PSUM") as psp,
      ):
            # ---- resident constants/weights ----
            identb = wpool.tile([P, P], BF16)
            make_identity(nc, identb)
            identf = wpool.tile([P, P], F32, name="identf")
            make_identity(nc, identf)
            ones_b = wpool.tile([1, P], BF16, name="ones_b")
            nc.vector.memset(ones_b, 1.0)
            magict = wpool.tile([P, 1], I32)
            nc.vector.memset(magict, RSQRT_MAGIC)

            w1s = []
            for kc in range(HID // P):
                w = wpool.tile([P, DH], BF16, tag=f"w1_{kc}")
                nc.sync.dma_start(out=w, in_=w1_d[kc * P:(kc + 1) * P, :])
                w1s.append(w)
            w2s = []
            for kc in range(DH // P):
                w = wpool.tile([P, DH], BF16, tag=f"w2_{kc}")
                nc.sync.dma_start(out=w, in_=w2_d[kc * P:(kc + 1) * P, :])
                w2s.append(w)
            w3s = []
            for kc in range(DH // P):
                w = wpool.tile([P, HID], BF16, tag=f"w3_{kc}")
                nc.sync.dma_start(out=w, in_=w3_d[kc * P:(kc + 1) * P, :])
                w3s.append(w)
            r1t = wpool.tile([1, n_steps, DH], BF16, name="r1t")
            nc.sync.dma_start(out=r1t, in_=r1_d)
            b2s = b3s = voff_s = None
            if use_b2:
                b2s = wpool.tile([1, DH], BF16)
                nc.sync.dma_start(out=b2s, in_=b2_d)
            if use_b3:
                b3s = wpool.tile([1, HID], BF16)
                nc.sync.dma_start(out=b3s, in_=b3_d)
            if use_voff:
                voff_s = wpool.tile([1, vocab], BF16)
                nc.sync.dma_start(out=voff_s, in_=voff_d)
            gbs = None
            if gb_d is not None:
                gbs = wpool.tile([P, 4, DH], F32)
                nc.sync.dma_start(out=gbs, in_=gb_d.to_broadcast([P, 4, DH]))

            hs = []
            for tp in range(tp_n):
                h = wpool.tile([P, HID], F32, tag=f"h_{tp}")
                nc.sync.dma_start(out=h, in_=h0_d[tp * P:(tp + 1) * P, :])
                hs.append(h)
            hcTs = [wpool.tile([P, HID], BF16, tag=f"hcT_{tp}",
                               name=f"hcT_{tp}") for tp in range(tp_n)]

            def rsqrt_dve(y, u, tmp, n_iter=1):
                """y = 1/sqrt(u) via bit-trick + Newton, all on DVE."""
                nc.vector.tensor_scalar(out=y.bitcast(I32),
                                        in0=u.bitcast(I32), scalar1=1,
                                        scalar2=None,
                                        op0=ALU.logical_shift_right)
                nc.vector.tensor_tensor(out=y.bitcast(I32), in0=magict,
                                        in1=y.bitcast(I32), op=ALU.subtract)
                for _ in range(n_iter):
                    nc.vector.tensor_tensor(out=tmp, in0=y, in1=y,
                                            op=ALU.mult)
                    nc.vector.tensor_tensor(out=tmp, in0=tmp, in1=u,
                                            op=ALU.mult)
                    nc.vector.tensor_scalar(out=tmp, in0=tmp, scalar1=-0.5,
                                            scalar2=1.5, op0=ALU.mult,
                                            op1=ALU.add)
                    nc.vector.tensor_tensor(out=y, in0=y, in1=tmp,
                                            op=ALU.mult)

            def ln_stats(pps):
                """Mean/var over the PSUM halves -> (rstd, -mu*rstd) [P,1]."""
                n = len(pps)
                st = work.tile([P, n, 6], F32, tag="st")
                for i, pp in enumerate(pps):
                    nc.vector.bn_stats(out=st[:, i, :], in_=pp)
                mv = work.tile([P, 2], F32, tag="mv")
                nc.vector.bn_aggr(out=mv, in_=st)
                sc = work.tile([P, 4], F32, tag="sc")
                u, rstd, tmp, nbias = (sc[:, 0:1], sc[:, 1:2], sc[:, 2:3],
                                       sc[:, 3:4])
                nc.vector.tensor_scalar(out=u, in0=mv[:, 1:2], scalar1=EPS,
                                        scalar2=None, op0=ALU.add)
                rsqrt_dve(rstd, u, tmp)
                nc.vector.tensor_scalar(out=nbias, in0=mv[:, 0:1],
                                        scalar1=-1.0, scalar2=rstd,
                                        op0=ALU.mult, op1=ALU.mult)
                return rstd, nbias

            def gelu_evac(dst, pps, rstd, nbias, gb_idx):
                """dst[bf16 SBUF] <- gelu(LN-normalized PSUM halves)."""
                if gb_idx is None or gbs is None:
                    for i, pp in enumerate(pps):
                        nc.scalar.activation(out=dst[:, ds(i * 512, 512)],
                                             in_=pp, func=AF.Gelu,
                                             scale=rstd, bias=nbias)
                else:
                    for i, pp in enumerate(pps):
                        nc.scalar.activation(out=dst[:, ds(i * 512, 512)],
                                             in_=pp, func=AF.Identity,
                                             scale=rstd, bias=nbias)
                    width = 512 * len(pps)
                    g_t, be_t = gbs[:, gb_idx, :], gbs[:, gb_idx + 1, :]
                    nc.vector.tensor_mul(out=dst, in0=dst,
                                         in1=g_t[:, :width])
                    nc.vector.tensor_add(out=dst, in0=dst,
                                         in1=be_t[:, :width])
                    for i in range(len(pps)):
                        nc.scalar.activation(out=dst[:, ds(i * 512, 512)],
                                             in_=dst[:, ds(i * 512, 512)],
                                             func=AF.Gelu)

            def transpose_hT(h):
                """SBUF bf16 [P,HID] <- 128-block transposes of f32 h."""
                pt = psp.tile([P, HID], F32, tag="ps")
                for c in range(HID // P):
                    nc.tensor.transpose(pt[:, ts(c, P)], h[:, ts(c, P)],
                                        identf)
                hT = work.tile([P, HID], BF16, tag="hT")
                nc.vector.tensor_copy(out=hT, in_=pt)
                return hT

            def transpose_zT(src):
                """SBUF bf16 [P,DH] <- 128-block transposes of bf16 src."""
                pt = psp.tile([P, DH], BF16, tag="pT", bufs=2)
                for c in range(DH // P):
                    nc.tensor.transpose(pt[:, ts(c, P)], src[:, ts(c, P)],
                                        identb)
                out = work.tile([P, DH], BF16, tag="zT")
                nc.scalar.copy(out=out, in_=pt)
                return out

            def mlp_layer(xT, ws, n_k, extra):
                """PSUM halves of xT.T @ W (+ optional K=1 extra rows)."""
                pps = []
                for i in range(2):
                    sl = ds(i * 512, 512)
                    pp = psp.tile([P, 512], F32, tag="ps")
                    for kc in range(n_k):
                        nc.tensor.matmul(pp, xT[:, ts(kc, P)], ws[kc][:, sl],
                                         start=(kc == 0),
                                         stop=(kc == n_k - 1 and not extra))
                    ex = extra or []
                    for j, (row, tab) in enumerate(ex):
                        nc.tensor.matmul(pp, row, tab[:, sl], start=False,
                                         stop=(j == len(ex) - 1))
                    pps.append(pp)
                return pps

            # ================= diffusion =================
            for step in range(n_steps):
                c_isa = float(coef[step] * isa[step])
                isa_f = float(isa[step])
                r1step = r1t[:, step, :]
                z1s, z2s = {}, {}
                for tp in range(tp_n):
                    hT = transpose_hT(hs[tp])
                    pps = mlp_layer(hT, w1s, 4, [(ones_b, r1step)])
                    rstd, nb = ln_stats(pps)
                    z1 = work.tile([P, DH], BF16, tag="zn", name=f"z1_{tp}")
                    gelu_evac(z1, pps, rstd, nb, 0 if apply_gb1 else None)
                    z1s[tp] = z1
                for tp in range(tp_n):
                    z1T = transpose_zT(z1s[tp])
                    extra2 = [(ones_b, b2s)] if use_b2 else None
                    pps = mlp_layer(z1T, w2s, 8, extra2)
                    rstd, nb = ln_stats(pps)
                    z2 = work.tile([P, DH], BF16, tag="zn", name=f"z2_{tp}")
                    gelu_evac(z2, pps, rstd, nb, 2 if apply_gb2 else None)
                    z2s[tp] = z2
                for tp in range(tp_n):
                    z2T = transpose_zT(z2s[tp])
                    ps3 = psp.tile([P, 512], F32, tag="ps")
                    for kc in range(8):
                        last = (kc == 7) and not use_b3
                        nc.tensor.matmul(ps3, z2T[:, ts(kc, P)], w3s[kc],
                                         start=(kc == 0), stop=last)
                    if use_b3:
                        nc.tensor.matmul(ps3, ones_b, b3s, start=False,
                                         stop=True)
                    # h = isa*h - (coef*isa)*score
                    h = hs[tp]
                    sc = work.tile([P, HID], F32, tag="sch")
                    nc.vector.tensor_scalar(out=sc, in0=ps3, scalar1=c_isa,
                                            scalar2=None, op0=ALU.mult)
                    hm = work.tile([P, HID], F32, tag="hm")
                    nc.scalar.mul(hm, h, isa_f)
                    nc.gpsimd.tensor_tensor(out=h, in0=hm, in1=sc,
                                            op=ALU.subtract)

            # ============ final LN + hcT (bf16) ============
            for tp in range(tp_n):
                h = hs[tp]
                st = work.tile([P, 6], F32, tag="stf")
                nc.vector.bn_stats(out=st, in_=h)
                mv = work.tile([P, 2], F32, tag="mv")
                nc.vector.bn_aggr(out=mv, in_=st)
                sc = work.tile([P, 4], F32, tag="sc")
                u, rstd, tmp, nbias = (sc[:, 0:1], sc[:, 1:2], sc[:, 2:3],
                                       sc[:, 3:4])
                nc.vector.tensor_scalar(out=u, in0=mv[:, 1:2], scalar1=EPS,
                                        scalar2=None, op0=ALU.add)
                rsqrt_dve(rstd, u, tmp, n_iter=2)
                nc.vector.tensor_scalar(out=nbias, in0=mv[:, 0:1],
                                        scalar1=-1.0, scalar2=rstd,
                                        op0=ALU.mult, op1=ALU.mult)
                hc = work.tile([P, HID], BF16, tag="hcb")
                nc.scalar.activation(out=hc, in_=h, func=AF.Identity,
                                     scale=rstd, bias=nbias)
                pt = psp.tile([P, DH], BF16, tag="pT", bufs=2)
                for kc in range(HID // P):
                    nc.tensor.transpose(pt[:, ts(kc, P)], hc[:, ts(kc, P)],
                                        identb)
                nc.vector.tensor_copy(out=hcTs[tp], in_=pt[:, :HID])

            # ================= logits (bf16) =================
            VC = 2048  # vocab stream chunk
            n_vc = (vocab + VC - 1) // VC
            n_out = 0
            for vc in range(n_vc):
                v0 = vc * VC
                vn = min(VC, vocab - v0)
                et = embp.tile([P, 4, vn], BF16, tag="et")
                for kc in range(4):
                    nc.sync.dma_start(
                        out=et[:, kc, :],
                        in_=emb_d[kc * P:(kc + 1) * P, v0:v0 + vn])
                for tp in range(tp_n):
                    for i in range((vn + 511) // 512):
                        w = min(512, vn - i * 512)
                        pl = psp.tile([P, 512], F32, tag="ps")
                        for kc in range(4):
                            last = (kc == 3) and not use_voff
                            nc.tensor.matmul(
                                pl[:, :w], hcTs[tp][:, ts(kc, P)],
                                et[:, kc, ds(i * 512, w)],
                                start=(kc == 0), stop=last)
                        if use_voff:
                            nc.tensor.matmul(
                                pl[:, :w], ones_b,
                                voff_s[:, ds(v0 + i * 512, w)],
                                start=False, stop=True)
                        lo = loutp.tile([P, 512], BF16, tag="lo")
                        if n_out % 2 == 0:
                            nc.vector.tensor_copy(out=lo[:, :w], in_=pl[:, :w])
                        else:
                            nc.scalar.copy(out=lo[:, :w], in_=pl[:, :w])
                        n_out += 1
                        nc.sync.dma_start(
                            out=out_d[tp * P:(tp + 1) * P,
                                      v0 + i * 512:v0 + i * 512 + w],
                            in_=lo[:, :w])
    nc.compile()
    return nc


def host_prep(x, embed, W1, b1, g1, be1, W2, b2, g2, be2, W3, b3, gn, bn,
              n_steps=N_STEPS):
    """Pure-numpy input prep shared by all cores."""
    BF = ml_dtypes.bfloat16
    x = np.asarray(x).reshape(-1)
    embed = np.asarray(embed, dtype=np.float32)
    W1 = np.asarray(W1, dtype=np.float32)
    b1 = np.asarray(b1, dtype=np.float32)
    t_norm, _, _ = _step_consts(n_steps)
    h0 = embed[x]                                     # [T_total, HID] f32
    r1 = (t_norm[:, None] * W1[HID][None, :]
          + b1[None, :]).astype(BF)[None]
    gnf = np.asarray(gn, dtype=np.float32)
    embt = np.ascontiguousarray(
        (embed * gnf[None, :]).T.astype(BF))          # [HID, VOCAB]
    voff = (np.asarray(bn, dtype=np.float32) @ embed.T).astype(np.float32)
    return dict(
        h0=h0,
        w1=np.ascontiguousarray(W1[:HID]).astype(BF),
        r1=r1,
        w2=np.asarray(W2, dtype=np.float32).astype(BF),
        w3=np.asarray(W3, dtype=np.float32).astype(BF),
        embt=embt,
        b2=np.asarray(b2, dtype=np.float32).reshape(1, -1).astype(BF),
        b3=np.asarray(b3, dtype=np.float32).reshape(1, -1).astype(BF),
        voff=voff.reshape(1, -1).astype(BF),
        g1=np.asarray(g1, dtype=np.float32),
        be1=np.asarray(be1, dtype=np.float32),
        g2=np.asarray(g2, dtype=np.float32),
        be2=np.asarray(be2, dtype=np.float32),
    )


_CACHE = {}


def _get_program(key, **kw):
    if key not in _CACHE:
        _CACHE[key] = build_program(**kw)
    return _CACHE[key]


def kernel(x, embed, W1, b1, g1, be1, W2, b2, g2, be2, W3, b3, gn, bn,
           run_kwargs=None):
    pre = host_prep(x, embed, W1, b1, g1, be1, W2, b2, g2, be2, W3, b3,
                    gn, bn)

    apply_gb1 = bool(np.any(pre["g1"] != 1.0) or np.any(pre["be1"] != 0.0))
    apply_gb2 = bool(np.any(pre["g2"] != 1.0) or np.any(pre["be2"] != 0.0))
    use_b2 = bool(np.any(pre["b2"]))
    use_b3 = bool(np.any(pre["b3"]))
    use_voff = bool(np.any(pre["voff"]))

    key = (apply_gb1, apply_gb2, use_b2, use_b3, use_voff)
    nc = _get_program(key, apply_gb1=apply_gb1, apply_gb2=apply_gb2,
                      use_b2=use_b2, use_b3=use_b3, use_voff=use_voff)

    common = {"w1": pre["w1"], "r1": pre["r1"], "w2": pre["w2"],
              "w3": pre["w3"], "embt": pre["embt"]}
    if use_b2:
        common["b2"] = pre["b2"]
    if use_b3:
        common["b3"] = pre["b3"]
    if use_voff:
        common["voff"] = pre["voff"]
    if apply_gb1 or apply_gb2:
        common["gb"] = np.stack([pre["g1"], pre["be1"], pre["g2"],
                                 pre["be2"]])

    in_maps = []
    for c in range(N_CORES):
        m = dict(common)
        m["h0"] = np.ascontiguousarray(pre["h0"][c * T_CORE:(c + 1) * T_CORE])
        in_maps.append(m)

    res = bass_utils.run_bass_kernel_spmd(
        nc, in_maps, core_ids=list(range(N_CORES)), **(run_kwargs or {}))
    out = np.concatenate(
        [np.asarray(res.results[c]["logits"]) for c in range(N_CORES)],
        axis=0).astype(np.float32)
    kernel.last_results = res
    return out.reshape(B, S, VOCAB)
